# revision 25
# baseline (speedup 1.0000x reference)
"""AUGRU cell (attention-gated GRU update) on 8 Trainium2 NeuronCores.

Data-parallel: the batch dim (16384) of x / att_score / hidden is sharded
across 8 cores (2048 rows each); the six 512x512 weight matrices are
replicated.

Per-core dataflow (per 128-row batch tile, 16 tiles):
  zr = x @ W_r + h @ U_r          (PSUM accum, 8 matmuls)
  hu = h @ U_h ; xh = x @ W_h
  zu = x @ W_u + h @ U_u          (update gate last: shortest tail)
  r = sigmoid(zr); g = tanh(r * hu + xh); d = g - h
  u2 = att * sigmoid(zu)
  out = h + u2 * d                (== (1-u2)*h + u2*g)

Group order zr, hu, xh, zu means everything except the short
sigmoid(zu) -> u2 -> u2*d -> +h chain completes while the zu matmuls
still run; the last tile runs that chain in two H-halves to pipeline
ACT/DVE and cut the kernel tail.

Matmuls in bf16 (rel err ~2.4e-3 vs the 2e-2 gate). All matmul operands
are HOST-PREPACKED into DRAM buffers whose per-partition bytes are
contiguous in exactly the SBUF-resident layout, so every load is one
direct DMA (128 descriptors x 2-4KB) - no staging copies, no DVE casts.
DMAs are spread across the sync/scalar/gpsimd/vector engine queues so
weight and batch-chunk loads issue in parallel right after the framework
preamble; junk bf16 ldweights keep the PE busy (HAM warm) while the
first weights land. Each PSUM bank keeps a single releasing engine
(zu/zr: ACT sigmoid; hu/xh: DVE) so per-Matmult sync waits stay <=1;
stragglers are legalized by _split_multi_waits.
"""

import os
import sys

if "/opt/trn_rl_repo" not in sys.path:
    sys.path.insert(0, "/opt/trn_rl_repo")

import numpy as np

NCORES = 8
P = 128
MM_DTYPE = os.environ.get("MM_DTYPE", "bf16")  # "bf16" or "f32r"


def _bchunks(Bc):
    """Batch-chunk widths for the x/h loads: a small first chunk covering
    the software-pipelined first two tiles, then wide chunks."""
    ws, rem = [], Bc
    for w in (2 * P, 2 * P):
        if rem <= 0:
            break
        w = min(w, rem)
        ws.append(w)
        rem -= w
    while rem > 0:
        w = min(4 * P, rem)
        ws.append(w)
        rem -= w
    return ws

_PROGRAM_CACHE = {}


def _split_multi_waits(nc):
    """walrus codegen accepts at most ONE sync wait per instruction (the
    TPB EVENTS struct has a single wait slot and setupSyncWait refuses to
    spill).  Tile's add_semaphores can emit several waits on one
    instruction; hoist all but the last into same-engine no-ops inserted
    immediately before it.  The engine executes the no-ops (each blocking
    on one semaphore) then the instruction - identical semantics.

    Matmult/Ldweights get ALL waits hoisted: a wait carried on a PE
    instruction breaks the fill/drain overlap with the previous matmul
    (~210ns per occurrence, once per tile); a NoOp carrying the wait
    dispatches while the previous matmul still streams, so the pipeline
    stays full."""
    import concourse.mybir as mybir

    for fn in nc.m.functions:
        for blk in fn.blocks:
            insts = blk.instructions
            i = 0
            while i < len(insts):
                inst = insts[i]
                si = inst.sync_info
                nhoist = 0
                if si is not None and si.on_wait:
                    if type(inst).__name__ in ("InstMatmult", "InstLdweights"):
                        nhoist = len(si.on_wait)
                    elif len(si.on_wait) > 1:
                        nhoist = len(si.on_wait) - 1
                if nhoist:
                    waits = list(si.on_wait)
                    inst.sync_info = mybir.SyncInfo(
                        on_wait=waits[nhoist:], on_update=list(si.on_update)
                    )
                    for j, w in enumerate(waits[:nhoist]):
                        nop = mybir.InstNoOp(
                            name=nc.get_next_instruction_name(),
                            sync_info=mybir.SyncInfo(on_wait=[w], on_update=[]),
                            bass_nofuse=True,
                            engine=inst.engine,
                        )
                        nc.register_instruction(nop)
                        insts.insert(i + j, nop)
                    i += nhoist
                i += 1


def _build_program(D, H, Bc, with_bias, mm_dtype=None):
    import concourse.bass as bass
    import concourse.mybir as mybir
    import concourse.tile as tile
    from concourse.alu_op_type import AluOpType

    f32 = mybir.dt.float32
    bf16_mode = (mm_dtype or MM_DTYPE) == "bf16"
    mm_dt = mybir.dt.bfloat16 if bf16_mode else mybir.dt.float32r
    Sig = mybir.ActivationFunctionType.Sigmoid
    Tanh = mybir.ActivationFunctionType.Tanh

    KD = D // P  # K chunks for x-side matmuls
    KH = H // P  # K chunks for h-side matmuls
    TILES = Bc // P

    nc = bass.Bass()
    # Host-prepacked DRAM layouts: per-partition bytes contiguous, matching
    # the SBUF-resident tiles exactly (one fat descriptor per partition).
    xT_p = nc.declare_dram_parameter("xT", [P, KD * Bc], mm_dt, isOutput=False)
    hT_p = nc.declare_dram_parameter("hT", [P, KH * Bc], mm_dt, isOutput=False)
    hN_p = nc.declare_dram_parameter("hN", [Bc, H], mybir.dt.bfloat16, isOutput=False)
    att_p = nc.declare_dram_parameter("att", [P, TILES], f32, isOutput=False)
    wnames = ("wu", "wr", "wh", "uu", "ur", "uh")
    w_p = {n: nc.declare_dram_parameter(n, [P, (KD if n[0] == "w" else KH) * H],
                                        mm_dt, isOutput=False) for n in wnames}
    if with_bias:
        b_p = {n: nc.declare_dram_parameter(n, [P, H], f32, isOutput=False)
               for n in ("bub", "brb", "bhb")}
    out_p = nc.declare_dram_parameter("out", [Bc, H], f32, isOutput=True)

    wview = {n: w_p[n][:].rearrange("ki (ko h) -> ki ko h",
                                    ko=KD if n[0] == "w" else KH) for n in wnames}

    CH_W = _bchunks(Bc)

    with tile.TileContext(nc) as tc:
        with (
            tc.tile_pool(name="w", bufs=1) as wpool,
            tc.tile_pool(name="dat", bufs=4) as dpool,
            tc.tile_pool(name="ep", bufs=3) as epool,
            tc.tile_pool(name="ps", bufs=2, space="PSUM") as ppool,
        ):
            w_sb = {n: wpool.tile([P, KD if n[0] == "w" else KH, H], mm_dt,
                                  tag=n, name=f"w_{n}") for n in wnames}
            # One resident tile per batch chunk: DMA src AND dst are then
            # contiguous per partition (2-4KB descriptors; slicing one big
            # [P, KD, Bc] tile gives 512B descriptors and ~1/4 the DMA rate).
            xc_sb = [wpool.tile([P, KD, w], mm_dt, tag=f"xc{c}", name=f"xc{c}")
                     for c, w in enumerate(CH_W)]
            hc_sb = [wpool.tile([P, KH, w], mm_dt, tag=f"hc{c}", name=f"hc{c}")
                     for c, w in enumerate(CH_W)]
            att_sb = wpool.tile([P, TILES], f32, tag="att")

            # tile index -> (chunk, local column offset)
            t2c = {}
            lo = 0
            for c, w in enumerate(CH_W):
                for tt in range(w // P):
                    t2c[(lo + tt * P) // P] = (c, tt * P)
                lo += w

            def operand(t, ki, side):
                c, off = t2c[t]
                sb = xc_sb[c] if side == "x" else hc_sb[c]
                return sb[:, ki, off:off + P]

            # PE warm-up: the HAM clock gate needs ~3.4us of sustained PE
            # activity before it lifts the array clock to 2.4 GHz. Junk
            # bf16 weight loads keep the PE busy while the first DMAs
            # land, so the real matmuls start warm. memset on gpsimd so
            # the LDWs are not gated behind any DMA-issuing engine.
            warm = wpool.tile([P, P], mybir.dt.bfloat16, tag="warm")
            nc.gpsimd.memset(warm, 0.0)
            for _ in range(20):
                nc.tensor.ldweights(warm)

            # Direct DMAs, spread across the three HWDGE queues (sync,
            # scalar, gpsimd), in consumption order. The first two tiles
            # run their zr/hu groups before any xh/zu (see below), so the
            # early need order is xc0, wr, hc0, ur, uh, then wh, wu, uu.
            #   sync:   x chunk0, wr halves, wu, att   then out stores
            #   scalar: ur halves, wh   then per-tile hN loads + ACTs
            #   gpsimd: h chunk0, uh halves, x/h chunks 1.., uu
            def chunk_dma(eng, sbs, view, c):
                lo = sum(CH_W[:c])
                KO = sbs[c].shape[1]
                src = view[:, KO * lo:KO * (lo + CH_W[c])].rearrange(
                    "ki (ko b) -> ki ko b", ko=KO)
                eng.dma_start(sbs[c], src)

            def w_dma(eng, n, piece, npieces):
                KO = w_sb[n].shape[1]
                w = KO // npieces
                sl = slice(piece * w, (piece + 1) * w)
                eng.dma_start(w_sb[n][:, sl], wview[n][:, sl])

            # wr/ur in ko-quarters so the PE can start on the first
            # quarter and consume each piece as it lands.
            w_dma(nc.sync, "wr", 0, 4)
            w_dma(nc.scalar, "ur", 0, 4)
            chunk_dma(nc.gpsimd, xc_sb, xT_p[:], 0)
            w_dma(nc.sync, "wr", 1, 4)
            w_dma(nc.scalar, "ur", 1, 4)
            chunk_dma(nc.gpsimd, hc_sb, hT_p[:], 0)
            w_dma(nc.sync, "wr", 2, 4)
            w_dma(nc.scalar, "ur", 2, 4)
            w_dma(nc.sync, "wr", 3, 4)
            w_dma(nc.scalar, "ur", 3, 4)
            w_dma(nc.gpsimd, "uh", 0, 2)
            w_dma(nc.gpsimd, "uh", 1, 2)
            nc.scalar.dma_start(w_sb["wh"], wview["wh"])
            nc.sync.dma_start(w_sb["wu"], wview["wu"])
            nc.sync.dma_start(att_sb, att_p[:])
            nc.gpsimd.dma_start(w_sb["uu"], wview["uu"])
            for c in range(1, len(CH_W)):
                chunk_dma(nc.gpsimd, xc_sb, xT_p[:], c)
                chunk_dma(nc.gpsimd, hc_sb, hT_p[:], c)
            if with_bias:
                b_sb = {}
                for n in ("bub", "brb", "bhb"):
                    t = wpool.tile([P, H], f32, tag=n)
                    nc.scalar.dma_start(t, b_p[n][:])
                    b_sb[n] = t

            bf16 = mybir.dt.bfloat16
            psum = {}

            GROUP_W = {"zr": ("wr", "ur"), "zu": ("wu", "uu"),
                       "hu": (None, "uh"), "xh": ("wh", None)}

            def mm_piece(t, name, side, kis):
                """Emit the matmuls of group `name` for tile t restricted
                to `kis` of `side` ('x'/'h'). start/stop flags derive from
                the group's overall first/last matmul."""
                wx, wh_ = GROUP_W[name]
                pt = psum[name]
                first_side = "x" if wx else "h"
                last_side = "h" if wh_ else "x"
                K = KD if side == "x" else KH
                wn = wx if side == "x" else wh_
                for ki in kis:
                    nc.tensor.matmul(pt, operand(t, ki, side), w_sb[wn][:, ki],
                                     start=side == first_side and ki == 0,
                                     stop=side == last_side and ki == K - 1)

            def new_group(t, name):
                psum[name] = ppool.tile([P, H], f32, tag=name,
                                        name=f"p_{name}_{t}")

            def mm_groups(t, names):
                for name in names:
                    new_group(t, name)
                    wx, wh_ = GROUP_W[name]
                    if wx:
                        mm_piece(t, name, "x", range(KD))
                    if wh_:
                        mm_piece(t, name, "h", range(KH))

            def epilogue(t, saved=None):
                bsl = slice(t * P, (t + 1) * P)
                h_t = dpool.tile([P, H], bf16, tag="h")
                nc.scalar.dma_start(h_t, hN_p[bsl, :])
                ps = saved or psum
                p_zr, p_hu, p_xh, p_zu = (ps[n] for n in ("zr", "hu", "xh", "zu"))

                # PSUM releasing engines: zr/zu by ACT sigmoid, hu/xh by
                # DVE. Group order zr, hu, xh, zu means the candidate
                # chain (r, r*hu+xh, tanh, -h) completes while the zu
                # matmuls still run; only sig(zu) -> stt -> +h trails the
                # last matmul.
                u = epool.tile([P, H], f32, tag="u")
                r = epool.tile([P, H], f32, tag="r")
                g = epool.tile([P, H], f32, tag="g")
                d = epool.tile([P, H], f32, tag="d")
                o = epool.tile([P, H], f32, tag="o")
                if with_bias:
                    zus = epool.tile([P, H], f32, tag="zus")
                    zrs = epool.tile([P, H], f32, tag="zrs")
                att_c = att_sb[:, t:t + 1]

                if with_bias:
                    nc.vector.tensor_add(zrs, p_zr, b_sb["brb"])
                    nc.scalar.activation(r, zrs, Sig)
                else:
                    nc.scalar.activation(r, p_zr, Sig)
                nc.vector.tensor_mul(g, r, p_hu)       # r * (h @ U_h)
                nc.vector.tensor_add(g, g, p_xh)       # + x @ W_h
                if with_bias:
                    nc.vector.tensor_add(g, g, b_sb["bhb"])
                nc.scalar.activation(g, g, Tanh)       # hhat
                nc.vector.tensor_sub(d, g, h_t)        # hhat - h
                if with_bias:
                    nc.vector.tensor_add(zus, p_zu, b_sb["bub"])
                    nc.scalar.activation(u, zus, Sig)
                else:
                    nc.scalar.activation(u, p_zu, Sig)
                # m = (u * att) * d, fused on DVE
                nc.vector.scalar_tensor_tensor(d, u, att_c, d,
                                               AluOpType.mult, AluOpType.mult)
                nc.vector.tensor_add(o, d, h_t)        # h + u2*(hhat-h)
                nc.sync.dma_start(out_p[bsl, :], o)

            # Software-pipelined start: tiles 0/1 are emitted one
            # weight-piece at a time across both tiles, in the exact
            # order the weight DMAs land (wr halves, ur halves, uh
            # halves, wh, wu, uu). The PE starts on the first quarter
            # weight and consumes each piece as it arrives instead of
            # stalling on any one tile's full weight set.
            if TILES >= 2:
                saved = [{} for _ in range(2)]
                for t in (0, 1):
                    new_group(t, "zr")
                    saved[t]["zr"] = psum["zr"]
                for ki in range(KD):          # wr arrives in ko-quarters
                    for t in (0, 1):
                        psum["zr"] = saved[t]["zr"]
                        mm_piece(t, "zr", "x", (ki,))
                for ki in range(KH):          # ur quarters
                    for t in (0, 1):
                        psum["zr"] = saved[t]["zr"]
                        mm_piece(t, "zr", "h", (ki,))
                hk = KH // 2
                for t in (0, 1):
                    new_group(t, "hu")
                    saved[t]["hu"] = psum["hu"]
                    mm_piece(t, "hu", "h", range(hk))
                for t in (0, 1):
                    psum["hu"] = saved[t]["hu"]
                    mm_piece(t, "hu", "h", range(hk, KH))
                for t in (0, 1):
                    new_group(t, "xh")
                    saved[t]["xh"] = psum["xh"]
                    mm_piece(t, "xh", "x", range(KD))
                for t in (0, 1):
                    new_group(t, "zu")
                    saved[t]["zu"] = psum["zu"]
                    mm_piece(t, "zu", "x", range(KD))
                for t in (0, 1):
                    psum["zu"] = saved[t]["zu"]
                    mm_piece(t, "zu", "h", range(KH))
                epilogue(0, saved[0])
                epilogue(1, saved[1])
                start = 2
            else:
                start = 0
            for t in range(start, TILES):
                mm_groups(t, ("zr", "hu", "xh", "zu"))
                epilogue(t)

    _split_multi_waits(nc)
    return nc


def check_waits(nc):
    """Matmults and Drains may carry at most 1 sync wait on walrus; other
    instruction classes tolerate more (walrus splits them itself)."""
    bad = []
    for fn in nc.m.functions:
        for blk in fn.blocks:
            for inst in blk.instructions:
                si = inst.sync_info
                nw = len(si.on_wait) if si else 0
                kind = type(inst).__name__
                if nw > 1:
                    bad.append((inst.name, kind, nw))
    return bad


def _get_program(D, H, Bc, with_bias):
    key = (D, H, Bc, with_bias, MM_DTYPE)
    if key not in _PROGRAM_CACHE:
        nc = _build_program(D, H, Bc, with_bias)
        bad = check_waits(nc)
        if bad:
            raise RuntimeError(f"instructions over the sync-wait limit: {bad}")
        _PROGRAM_CACHE[key] = nc
    return _PROGRAM_CACHE[key]


def _np32(a):
    return np.ascontiguousarray(np.asarray(a, dtype=np.float32))


def _bf16():
    import ml_dtypes

    return ml_dtypes.bfloat16


def _mm_np_dtype():
    return _bf16() if MM_DTYPE == "bf16" else np.float32


def _pack_bT(a, Bc, K, mmdt):
    """[Bc, K] activations -> [128, KO*Bc] with per-partition layout
    [chunk][ko][b_local] (chunk-major, matching the per-chunk DMAs)."""
    KO = K // P
    parts, lo = [], 0
    for w in _bchunks(Bc):
        blk = a[lo:lo + w].reshape(w, KO, P).transpose(2, 1, 0)  # [ki, ko, b]
        parts.append(blk.reshape(P, KO * w))
        lo += w
    return np.ascontiguousarray(np.concatenate(parts, axis=1).astype(mmdt))


def _pack_w(w, mmdt):
    """[K, H] weight -> [128, KO*H] with per-partition layout [ko, h]."""
    K, H = w.shape
    out = w.reshape(K // P, P, H).transpose(1, 0, 2)
    return np.ascontiguousarray(out.reshape(P, -1).astype(mmdt))


def _prepare(x, att_score, hidden, W_u, U_u, b_u, W_r, U_r, b_r, W_h, U_h, b_h):
    x = _np32(x)
    att_score = _np32(att_score)
    hidden = _np32(hidden)
    B, D = x.shape
    H = hidden.shape[1]
    assert B % (NCORES * P) == 0 and D % P == 0 and H % P == 0
    Bc = B // NCORES
    mmdt = _mm_np_dtype()

    weights = {
        "wu": _np32(W_u), "wr": _np32(W_r), "wh": _np32(W_h),
        "uu": _np32(U_u), "ur": _np32(U_r), "uh": _np32(U_h),
    }
    biases = [_np32(b_u), _np32(b_r), _np32(b_h)]
    with_bias = any(np.any(b) for b in biases)
    packed_w = {k: _pack_w(v, mmdt) for k, v in weights.items()}

    in_maps = []
    for c in range(NCORES):
        sl = slice(c * Bc, (c + 1) * Bc)
        xs, hs, at = x[sl], hidden[sl], att_score[sl]
        m = {
            "xT": _pack_bT(xs, Bc, D, mmdt),
            "hT": _pack_bT(hs, Bc, H, mmdt),
            "hN": np.ascontiguousarray(hs.astype(_bf16())),
            "att": np.ascontiguousarray(at.reshape(Bc // P, P).T),
        }
        m.update(packed_w)
        if with_bias:
            m["bub"] = np.ascontiguousarray(np.broadcast_to(biases[0], (P, H)))
            m["brb"] = np.ascontiguousarray(np.broadcast_to(biases[1], (P, H)))
            m["bhb"] = np.ascontiguousarray(np.broadcast_to(biases[2], (P, H)))
        in_maps.append(m)

    nc = _get_program(D, H, Bc, with_bias)
    return nc, in_maps


def _run(inputs, trace=False, **trace_kwargs):
    from concourse.bass_utils import run_bass_kernel_spmd

    nc, in_maps = _prepare(**inputs)
    res = run_bass_kernel_spmd(nc, in_maps, list(range(NCORES)), trace=trace,
                               **trace_kwargs)
    out = np.concatenate([res.results[i]["out"] for i in range(NCORES)], axis=0)
    return out, res


def kernel(**inputs):
    out, _ = _run(inputs, trace=False)
    return out
